# revision 11
# baseline (speedup 1.0000x reference)
"""BitLinearV2 Trainium2 kernel.

Computes: out = input @ (ternarize(weight, threshold) * scale[:, None]).T + bias
  input  [4, 2048, 4096] f32
  weight [11008, 4096] f32, threshold [11008, 1], scale [11008], bias [11008]
  out    [4, 2048, 11008] f32

Strategy: column-parallel over 8 NeuronCores (each core owns 1376 output
features).  Input (cast to bf16, transposed to feature-major) is replicated;
weights are ternarized on-device (ScalarE Abs/Sign, VectorE compare, GpSimd
mult) into a resident fp8 SBUF buffer ({-1,0,+1} are exact in fp8); the big
matmul accumulates over K=4096 in 32 PSUM passes; scale+bias applied to PSUM
in f32 on VectorE.

To hide the ternarize pipeline (~75us), the first RAMP_T token tiles' first
output slice accumulates k-outer in dedicated PSUM banks, giving the PE work
as each weight slab becomes ready.
"""

import numpy as np
import ml_dtypes

B, S, I, O = 4, 2048, 4096, 11008
T = B * S               # 8192 tokens
NCORES = 8
OSH = O // NCORES       # 1376 out features per core
KT = I // 128           # 32 contraction slabs
TT = T // 128           # 64 token tiles
O_SLICES = [(0, 512), (512, 512), (1024, 352)]
RAMP_T = 5              # token tiles whose j=0 slice accumulates during ternarize

_CACHE = {}


def _build_nc():
    import concourse.bass as bass
    import concourse.bacc as bacc
    import concourse.mybir as mybir
    import concourse.tile as tile

    nc = bacc.Bacc()
    x_d = nc.dram_tensor("x", [TT, 128, KT, 128], mybir.dt.bfloat16, kind="ExternalInput")
    w_d = nc.dram_tensor("w", [KT, 128, OSH], mybir.dt.float32, kind="ExternalInput")
    thr_d = nc.dram_tensor("thr", [128, OSH], mybir.dt.float32, kind="ExternalInput")
    scale_d = nc.dram_tensor("scale", [128, OSH], mybir.dt.float32, kind="ExternalInput")
    bias_d = nc.dram_tensor("bias", [128, OSH], mybir.dt.float32, kind="ExternalInput")
    out_d = nc.dram_tensor("out", [T, OSH], mybir.dt.float32, kind="ExternalOutput")

    AF = mybir.ActivationFunctionType
    ALU = mybir.AluOpType
    F32 = mybir.dt.float32
    BF16 = mybir.dt.bfloat16
    FP8 = mybir.dt.float8e4

    with tile.TileContext(nc) as tc:
        with (
            tc.tile_pool(name="consts", bufs=1) as consts,
            tc.tile_pool(name="wstage", bufs=2) as wstage,
            tc.tile_pool(name="tmps", bufs=2) as tmps,
            tc.tile_pool(name="xin", bufs=RAMP_T + 2) as xin,
            tc.tile_pool(name="oout", bufs=3) as oout,
            tc.tile_pool(name="psumA", bufs=1, space="PSUM") as psumA,
            tc.tile_pool(name="psum", bufs=1, space="PSUM") as psum,
        ):
            # per-out-feature vectors, pre-replicated across partitions on host
            thr_b = consts.tile([128, OSH], F32, tag="thr_b")
            scale_b = consts.tile([128, OSH], F32, tag="scale_b")
            bias_b = consts.tile([128, OSH], F32, tag="bias_b")
            nc.sync.dma_start(out=thr_b, in_=thr_d[:])
            nc.sync.dma_start(out=scale_b, in_=scale_d[:])
            nc.sync.dma_start(out=bias_b, in_=bias_d[:])

            # resident ternarized weights, feature-major, fp8 ({-1,0,1} exact)
            w_sb = consts.tile([128, KT, OSH], FP8, tag="w_sb")

            # input token tiles; first RAMP_T loaded up front for the ramp
            xts = {}
            for t in range(RAMP_T):
                xts[t] = xin.tile([128, KT, 128], BF16, tag="xt", name=f"xt{t}")
                nc.sync.dma_start(out=xts[t], in_=x_d[t])

            # phase A psum: j=0 slice of the first RAMP_T token tiles,
            # accumulated k-outer while ternarize streams slabs
            ptA = [
                psumA.tile([128, 512], F32, tag=f"ptA{t}", name=f"ptA{t}")
                for t in range(RAMP_T)
            ]

            for k in range(KT):
                # ternarize slab k: tern = sign(w) * (|w| >= thr)
                wf = wstage.tile([128, OSH], F32, tag="wf")
                nc.sync.dma_start(out=wf, in_=w_d[k])
                aw = tmps.tile([128, OSH], F32, tag="aw")
                nc.scalar.activation(aw, wf, AF.Abs)
                sg = tmps.tile([128, OSH], BF16, tag="sg")
                nc.scalar.activation(sg, wf, AF.Sign)
                m = tmps.tile([128, OSH], BF16, tag="m")
                nc.vector.tensor_tensor(m, aw, thr_b, ALU.is_ge)
                nc.gpsimd.tensor_tensor(w_sb[:, k, :], m, sg, ALU.mult)

                # phase A matmuls on the slab that just became ready
                o0, ow = O_SLICES[0]
                for t in range(RAMP_T):
                    nc.tensor.matmul(
                        ptA[t][:, :ow],
                        xts[t][:, k, :],
                        w_sb[:, k, o0 : o0 + ow],
                        start=(k == 0),
                        stop=(k == KT - 1),
                    )

            def epilogue(pt, t, j):
                o0, ow = O_SLICES[j]
                ot = oout.tile([128, 512], F32, tag=f"ot{j}", name=f"ot{j}", bufs=2)
                nc.vector.tensor_tensor(
                    ot[:, :ow], pt[:, :ow], scale_b[:, o0 : o0 + ow], ALU.mult
                )
                nc.vector.tensor_tensor(
                    ot[:, :ow], ot[:, :ow], bias_b[:, o0 : o0 + ow], ALU.add
                )
                nc.sync.dma_start(
                    out=out_d[t * 128 : (t + 1) * 128, o0 : o0 + ow], in_=ot[:, :ow]
                )

            for t in range(RAMP_T):
                epilogue(ptA[t], t, 0)

            # phases B+C: everything not covered by the ramp
            for t in range(TT):
                if t < RAMP_T:
                    xt = xts[t]
                    jlist = (1, 2)
                else:
                    xt = xin.tile([128, KT, 128], BF16, tag="xt", name="xt")
                    nc.sync.dma_start(out=xt, in_=x_d[t])
                    jlist = (0, 1, 2)
                for j in jlist:
                    o0, ow = O_SLICES[j]
                    pt = psum.tile([128, 512], F32, tag=f"pt{j}", name=f"pt{j}")
                    for k in range(KT):
                        nc.tensor.matmul(
                            pt[:, :ow],
                            xt[:, k, :],
                            w_sb[:, k, o0 : o0 + ow],
                            start=(k == 0),
                            stop=(k == KT - 1),
                        )
                    epilogue(pt, t, j)

    nc.compile()
    return nc


def _get_nc():
    if "nc" not in _CACHE:
        _CACHE["nc"] = _build_nc()
    return _CACHE["nc"]


def _run(inputs, trace=False, tmpdir=None):
    from concourse.bass_utils import run_bass_kernel_spmd

    x = np.asarray(inputs["input"], dtype=np.float32)
    w = np.asarray(inputs["weight"], dtype=np.float32)
    scale = np.asarray(inputs["scale"], dtype=np.float32)
    thr = np.asarray(inputs["threshold"], dtype=np.float32).reshape(O)
    bias = np.asarray(inputs["bias"], dtype=np.float32)

    # [T, I] -> [TT, p=128(feat), KT, tl=128(tok)] bf16, contiguous per partition
    x2 = x.reshape(T, I).astype(ml_dtypes.bfloat16)
    xh = np.ascontiguousarray(x2.reshape(TT, 128, KT, 128).transpose(0, 3, 2, 1))

    in_maps = []
    for c in range(NCORES):
        sl = slice(c * OSH, (c + 1) * OSH)
        wh = np.ascontiguousarray(w[sl].T).reshape(KT, 128, OSH)
        in_maps.append(
            {
                "x": xh,
                "w": wh,
                "thr": np.ascontiguousarray(np.broadcast_to(thr[sl], (128, OSH))),
                "scale": np.ascontiguousarray(np.broadcast_to(scale[sl], (128, OSH))),
                "bias": np.ascontiguousarray(np.broadcast_to(bias[sl], (128, OSH))),
            }
        )

    nc = _get_nc()
    res = run_bass_kernel_spmd(
        nc, in_maps, list(range(NCORES)), trace=trace, tmpdir=tmpdir
    )
    out = np.concatenate([res.results[c]["out"] for c in range(NCORES)], axis=1)
    return out.reshape(B, S, O), res


def kernel(**inputs) -> np.ndarray:
    out, _ = _run(inputs, trace=False)
    return out


# revision 13
# speedup vs baseline: 1.0203x; 1.0203x over previous
"""BitLinearV2 Trainium2 kernel.

Computes: out = input @ (ternarize(weight, threshold) * scale[:, None]).T + bias
  input  [4, 2048, 4096] f32
  weight [11008, 4096] f32, threshold [11008, 1], scale [11008], bias [11008]
  out    [4, 2048, 11008] f32

Strategy: column-parallel over 8 NeuronCores (each core owns 1376 output
features).  Input (cast to bf16, transposed to feature-major) is replicated;
weights are ternarized on-device (ScalarE Abs/Sign, VectorE compare, GpSimd
mult) into a resident fp8 SBUF buffer ({-1,0,+1} are exact in fp8); the big
matmul accumulates over K=4096 in 32 PSUM passes; scale+bias applied to PSUM
in f32 on VectorE.

To hide the ternarize pipeline (~75us), the first RAMP_T token tiles' first
output slice accumulates k-outer in dedicated PSUM banks, giving the PE work
as each weight slab becomes ready.
"""

import numpy as np
import ml_dtypes

B, S, I, O = 4, 2048, 4096, 11008
T = B * S               # 8192 tokens
NCORES = 8
OSH = O // NCORES       # 1376 out features per core
KT = I // 128           # 32 contraction slabs
TT = T // 128           # 64 token tiles
O_SLICES = [(0, 512), (512, 512), (1024, 352)]
RAMP_T = 5              # token tiles whose j=0 slice accumulates during ternarize

_CACHE = {}


def _build_nc():
    import concourse.bass as bass
    import concourse.bacc as bacc
    import concourse.mybir as mybir
    import concourse.tile as tile

    nc = bacc.Bacc()
    x_d = nc.dram_tensor("x", [TT, 128, KT, 128], mybir.dt.bfloat16, kind="ExternalInput")
    w_d = nc.dram_tensor("w", [KT, 128, OSH], mybir.dt.float32, kind="ExternalInput")
    thr_d = nc.dram_tensor("thr", [128, OSH], mybir.dt.float32, kind="ExternalInput")
    scale_d = nc.dram_tensor("scale", [128, OSH], mybir.dt.float32, kind="ExternalInput")
    bias_d = nc.dram_tensor("bias", [128, OSH], mybir.dt.float32, kind="ExternalInput")
    out_d = nc.dram_tensor("out", [T, OSH], mybir.dt.float32, kind="ExternalOutput")

    AF = mybir.ActivationFunctionType
    ALU = mybir.AluOpType
    F32 = mybir.dt.float32
    BF16 = mybir.dt.bfloat16
    FP8 = mybir.dt.float8e4

    with tile.TileContext(nc) as tc:
        with (
            tc.tile_pool(name="consts", bufs=1) as consts,
            tc.tile_pool(name="wstage", bufs=3) as wstage,
            tc.tile_pool(name="tmps", bufs=2) as tmps,
            tc.tile_pool(name="xin", bufs=RAMP_T + 2) as xin,
            tc.tile_pool(name="oout", bufs=3) as oout,
            tc.tile_pool(name="psumA", bufs=1, space="PSUM") as psumA,
            tc.tile_pool(name="psum", bufs=1, space="PSUM") as psum,
        ):
            # threshold vector (pre-replicated across partitions on host);
            # loaded first since the very first ternarize needs it
            thr_b = consts.tile([128, OSH], F32, tag="thr_b")
            nc.sync.dma_start(out=thr_b, in_=thr_d[:])

            # resident ternarized weights, feature-major, fp8 ({-1,0,1} exact)
            w_sb = consts.tile([128, KT, OSH], FP8, tag="w_sb")

            # phase A psum: j=0 slice of the first RAMP_T token tiles,
            # accumulated k-outer while ternarize streams slabs.  Group t
            # joins at slab t (so its x DMA can land progressively) and wraps
            # around to finish slabs 0..t-1 at the end.
            ptA = [
                psumA.tile([128, 512], F32, tag=f"ptA{t}", name=f"ptA{t}")
                for t in range(RAMP_T)
            ]
            xts = {}
            o0A, owA = O_SLICES[0]

            for k in range(KT):
                if k < RAMP_T:
                    xts[k] = xin.tile([128, KT, 128], BF16, tag="xt", name=f"xt{k}")
                    nc.sync.dma_start(out=xts[k], in_=x_d[k])

                # ternarize slab k: tern = sign(w) * (|w| >= thr)
                wf = wstage.tile([128, OSH], F32, tag="wf")
                nc.sync.dma_start(out=wf, in_=w_d[k])
                aw = tmps.tile([128, OSH], F32, tag="aw")
                nc.scalar.activation(aw, wf, AF.Abs)
                sg = tmps.tile([128, OSH], BF16, tag="sg")
                nc.scalar.activation(sg, wf, AF.Sign)
                m = tmps.tile([128, OSH], BF16, tag="m")
                nc.vector.tensor_tensor(m, aw, thr_b, ALU.is_ge)
                nc.vector.tensor_tensor(w_sb[:, k, :], m, sg, ALU.mult)

                # phase A matmuls on the slab that just became ready
                for t in range(min(k + 1, RAMP_T)):
                    nc.tensor.matmul(
                        ptA[t][:, :owA],
                        xts[t][:, k, :],
                        w_sb[:, k, o0A : o0A + owA],
                        start=(k == t),
                        stop=(k == KT - 1 and t == 0),
                    )

            # wrap-around: groups 1..RAMP_T-1 still owe slabs 0..t-1
            for t in range(1, RAMP_T):
                for k in range(t):
                    nc.tensor.matmul(
                        ptA[t][:, :owA],
                        xts[t][:, k, :],
                        w_sb[:, k, o0A : o0A + owA],
                        start=False,
                        stop=(k == t - 1),
                    )

            # scale/bias vectors are not needed until the first epilogue
            scale_b = consts.tile([128, OSH], F32, tag="scale_b")
            bias_b = consts.tile([128, OSH], F32, tag="bias_b")
            nc.sync.dma_start(out=scale_b, in_=scale_d[:])
            nc.sync.dma_start(out=bias_b, in_=bias_d[:])

            def epilogue(pt, t, j):
                o0, ow = O_SLICES[j]
                ot = oout.tile([128, 512], F32, tag=f"ot{j}", name=f"ot{j}", bufs=2)
                nc.vector.tensor_tensor(
                    ot[:, :ow], pt[:, :ow], scale_b[:, o0 : o0 + ow], ALU.mult
                )
                nc.vector.tensor_tensor(
                    ot[:, :ow], ot[:, :ow], bias_b[:, o0 : o0 + ow], ALU.add
                )
                nc.sync.dma_start(
                    out=out_d[t * 128 : (t + 1) * 128, o0 : o0 + ow], in_=ot[:, :ow]
                )

            for t in range(RAMP_T):
                epilogue(ptA[t], t, 0)

            # phases B+C: everything not covered by the ramp
            for t in range(TT):
                if t < RAMP_T:
                    xt = xts[t]
                    jlist = (1, 2)
                else:
                    xt = xin.tile([128, KT, 128], BF16, tag="xt", name="xt")
                    nc.sync.dma_start(out=xt, in_=x_d[t])
                    jlist = (0, 1, 2)
                for j in jlist:
                    o0, ow = O_SLICES[j]
                    pt = psum.tile([128, 512], F32, tag=f"pt{j}", name=f"pt{j}")
                    for k in range(KT):
                        nc.tensor.matmul(
                            pt[:, :ow],
                            xt[:, k, :],
                            w_sb[:, k, o0 : o0 + ow],
                            start=(k == 0),
                            stop=(k == KT - 1),
                        )
                    epilogue(pt, t, j)

    nc.compile()
    return nc


def _get_nc():
    if "nc" not in _CACHE:
        _CACHE["nc"] = _build_nc()
    return _CACHE["nc"]


def _run(inputs, trace=False, tmpdir=None):
    from concourse.bass_utils import run_bass_kernel_spmd

    x = np.asarray(inputs["input"], dtype=np.float32)
    w = np.asarray(inputs["weight"], dtype=np.float32)
    scale = np.asarray(inputs["scale"], dtype=np.float32)
    thr = np.asarray(inputs["threshold"], dtype=np.float32).reshape(O)
    bias = np.asarray(inputs["bias"], dtype=np.float32)

    # [T, I] -> [TT, p=128(feat), KT, tl=128(tok)] bf16, contiguous per partition
    x2 = x.reshape(T, I).astype(ml_dtypes.bfloat16)
    xh = np.ascontiguousarray(x2.reshape(TT, 128, KT, 128).transpose(0, 3, 2, 1))

    in_maps = []
    for c in range(NCORES):
        sl = slice(c * OSH, (c + 1) * OSH)
        wh = np.ascontiguousarray(w[sl].T).reshape(KT, 128, OSH)
        in_maps.append(
            {
                "x": xh,
                "w": wh,
                "thr": np.ascontiguousarray(np.broadcast_to(thr[sl], (128, OSH))),
                "scale": np.ascontiguousarray(np.broadcast_to(scale[sl], (128, OSH))),
                "bias": np.ascontiguousarray(np.broadcast_to(bias[sl], (128, OSH))),
            }
        )

    nc = _get_nc()
    res = run_bass_kernel_spmd(
        nc, in_maps, list(range(NCORES)), trace=trace, tmpdir=tmpdir
    )
    out = np.concatenate([res.results[c]["out"] for c in range(NCORES)], axis=1)
    return out.reshape(B, S, O), res


def kernel(**inputs) -> np.ndarray:
    out, _ = _run(inputs, trace=False)
    return out


# revision 16
# speedup vs baseline: 1.0416x; 1.0208x over previous
"""BitLinearV2 Trainium2 kernel.

Computes: out = input @ (ternarize(weight, threshold) * scale[:, None]).T + bias
  input  [4, 2048, 4096] f32
  weight [11008, 4096] f32, threshold [11008, 1], scale [11008], bias [11008]
  out    [4, 2048, 11008] f32

Strategy: column-parallel over 8 NeuronCores (each core owns 1376 output
features).  Input (cast to bf16, transposed to feature-major) is replicated;
weights are ternarized on-device into a resident fp8 SBUF buffer holding
2*tern in {-2,0,+2} (exact in fp8); the big matmul accumulates over K=4096
in 32 PSUM passes; 0.5*scale (halved on device) + bias applied to PSUM in
f32 on VectorE.

Ternarize uses the identity  tern = (sign(w - t) + sign(w + t)) / 2  for the
(always, per spec) uniform threshold t, with t passed as per-partition [128,1]
ScalarE bias vectors — no [128, OSH] threshold broadcast needed.  Slabs are
split between an ACT form (two Sign ops + VectorE add) and a DVE form (two
tensor_scalar compare ops + sub) to balance the two engines.  A non-uniform
threshold falls back to a general program.

To hide the ternarize pipeline, the first RAMP_T token tiles' first output
slice accumulates k-outer in dedicated PSUM banks (group t joins at slab t,
wrapping at the end), and dummy matmuls on a zero tile keep the PE busy/warm
during the initial weight-DMA head.
"""

import numpy as np
import ml_dtypes

B, S, I, O = 4, 2048, 4096, 11008
T = B * S               # 8192 tokens
NCORES = 8
OSH = O // NCORES       # 1376 out features per core
KT = I // 128           # 32 contraction slabs
TT = T // 128           # 64 token tiles
O_SLICES = [(0, 512), (512, 512), (1024, 352)]
RAMP_T = 5              # token tiles whose j=0 slice accumulates during ternarize
N_DUMMY = 80            # warm-up matmuls during the DMA head

_CACHE = {}


def _build_nc(uniform_thr=True):
    import concourse.bass as bass
    import concourse.bacc as bacc
    import concourse.mybir as mybir
    import concourse.tile as tile

    nc = bacc.Bacc()
    x_d = nc.dram_tensor("x", [TT, 128, KT, 128], mybir.dt.bfloat16, kind="ExternalInput")
    w_d = nc.dram_tensor("w", [KT, 128, OSH], mybir.dt.float32, kind="ExternalInput")
    # thrv columns: 0: -thr (sign-a bias), 1: +thr (sign-b bias),
    #               2: thr  (is_ge scalar), 3: -thr (is_le scalar)
    thrv_d = nc.dram_tensor("thrv", [128, 4], mybir.dt.float32, kind="ExternalInput")
    thr_d = nc.dram_tensor("thr", [128, OSH], mybir.dt.float32, kind="ExternalInput")
    scale_d = nc.dram_tensor("scale", [128, OSH], mybir.dt.float32, kind="ExternalInput")
    bias_d = nc.dram_tensor("bias", [128, OSH], mybir.dt.float32, kind="ExternalInput")
    out_d = nc.dram_tensor("out", [T, OSH], mybir.dt.float32, kind="ExternalOutput")

    AF = mybir.ActivationFunctionType
    ALU = mybir.AluOpType
    F32 = mybir.dt.float32
    BF16 = mybir.dt.bfloat16
    FP8 = mybir.dt.float8e4

    with tile.TileContext(nc) as tc:
        with (
            tc.tile_pool(name="consts", bufs=1) as consts,
            tc.tile_pool(name="wstage", bufs=3) as wstage,
            tc.tile_pool(name="tmps", bufs=2) as tmps,
            tc.tile_pool(name="xin", bufs=RAMP_T + 2) as xin,
            tc.tile_pool(name="oout", bufs=3) as oout,
            tc.tile_pool(name="psumA", bufs=1, space="PSUM") as psumA,
            tc.tile_pool(name="psum", bufs=1, space="PSUM") as psum,
        ):
            # resident 2x-ternarized weights, feature-major, fp8 ({-2,0,2} exact)
            w_sb = consts.tile([128, KT, OSH], FP8, tag="w_sb")

            thrv = consts.tile([128, 4], F32, tag="thrv")
            nc.sync.dma_start(out=thrv, in_=thrv_d[:])
            if not uniform_thr:
                thr_b = consts.tile([128, OSH], F32, tag="thr_b")
                nc.sync.dma_start(out=thr_b, in_=thr_d[:])

            # dummy warm-up matmuls: keep the PE active (HAM warm) while the
            # first weight slab DMA lands; results are discarded by the real
            # start=True accumulation below.
            dum = consts.tile([128, 512], BF16, tag="dum")
            nc.vector.memset(dum, 0.0)

            ptA = [
                psumA.tile([128, 512], F32, tag=f"ptA{t}", name=f"ptA{t}")
                for t in range(RAMP_T)
            ]
            for i in range(N_DUMMY):
                nc.tensor.matmul(
                    ptA[i % RAMP_T][:, :], dum[:, :128], dum[:, :],
                    start=True, stop=True,
                )

            # phase A: j=0 slice of the first RAMP_T token tiles, accumulated
            # k-outer while ternarize streams slabs.  Group t joins at slab t
            # (so its x DMA can land progressively) and wraps around to finish
            # slabs 0..t-1 at the end.
            xts = {}
            o0A, owA = O_SLICES[0]

            for k in range(KT):
                if k < RAMP_T:
                    xts[k] = xin.tile([128, KT, 128], BF16, tag="xt", name=f"xt{k}")
                    nc.sync.dma_start(out=xts[k], in_=x_d[k])

                # ternarize slab k into 2*tern in {-2, 0, +2}
                wf = wstage.tile([128, OSH], F32, tag="wf")
                nc.sync.dma_start(out=wf, in_=w_d[k])
                if not uniform_thr:
                    aw = tmps.tile([128, OSH], F32, tag="aw")
                    nc.scalar.activation(aw, wf, AF.Abs)
                    sg = tmps.tile([128, OSH], BF16, tag="sg")
                    nc.scalar.activation(sg, wf, AF.Sign)
                    m = tmps.tile([128, OSH], BF16, tag="m")
                    nc.vector.tensor_tensor(m, aw, thr_b, ALU.is_ge)
                    nc.vector.tensor_tensor(w_sb[:, k, :], m, sg, ALU.mult)
                elif k % 4 == 3:
                    # DVE form: (w >= t)*2 - (w <= -t)*2
                    ge2 = tmps.tile([128, OSH], BF16, tag="ge2")
                    nc.vector.tensor_scalar(
                        ge2, wf, thrv[:, 2:3], 2.0, ALU.is_ge, ALU.mult
                    )
                    le2 = tmps.tile([128, OSH], BF16, tag="le2")
                    nc.vector.tensor_scalar(
                        le2, wf, thrv[:, 3:4], 2.0, ALU.is_le, ALU.mult
                    )
                    nc.vector.tensor_tensor(w_sb[:, k, :], ge2, le2, ALU.subtract)
                else:
                    # ACT form: sign(w - t) + sign(w + t)
                    sa = tmps.tile([128, OSH], BF16, tag="sa")
                    nc.scalar.activation(sa, wf, AF.Sign, bias=thrv[:, 0:1])
                    sb = tmps.tile([128, OSH], BF16, tag="sb")
                    nc.scalar.activation(sb, wf, AF.Sign, bias=thrv[:, 1:2])
                    nc.vector.tensor_tensor(w_sb[:, k, :], sa, sb, ALU.add)

                # phase A matmuls on the slab that just became ready
                for t in range(min(k + 1, RAMP_T)):
                    nc.tensor.matmul(
                        ptA[t][:, :owA],
                        xts[t][:, k, :],
                        w_sb[:, k, o0A : o0A + owA],
                        start=(k == t),
                        stop=(k == KT - 1 and t == 0),
                    )

            # wrap-around: groups 1..RAMP_T-1 still owe slabs 0..t-1
            for t in range(1, RAMP_T):
                for k in range(t):
                    nc.tensor.matmul(
                        ptA[t][:, :owA],
                        xts[t][:, k, :],
                        w_sb[:, k, o0A : o0A + owA],
                        start=False,
                        stop=(k == t - 1),
                    )

            # scale/bias vectors are not needed until the first epilogue.
            # w_sb holds 2*tern, so halve the scale once on device.
            scale_b = consts.tile([128, OSH], F32, tag="scale_b")
            bias_b = consts.tile([128, OSH], F32, tag="bias_b")
            nc.sync.dma_start(out=scale_b, in_=scale_d[:])
            nc.sync.dma_start(out=bias_b, in_=bias_d[:])
            if uniform_thr:
                # w_sb holds 2*tern in the uniform-threshold program
                nc.vector.tensor_scalar_mul(scale_b, scale_b, 0.5)

            def epilogue(pt, t, j):
                o0, ow = O_SLICES[j]
                ot = oout.tile([128, 512], F32, tag=f"ot{j}", name=f"ot{j}", bufs=2)
                nc.vector.tensor_tensor(
                    ot[:, :ow], pt[:, :ow], scale_b[:, o0 : o0 + ow], ALU.mult
                )
                nc.vector.tensor_tensor(
                    ot[:, :ow], ot[:, :ow], bias_b[:, o0 : o0 + ow], ALU.add
                )
                nc.sync.dma_start(
                    out=out_d[t * 128 : (t + 1) * 128, o0 : o0 + ow], in_=ot[:, :ow]
                )

            for t in range(RAMP_T):
                epilogue(ptA[t], t, 0)

            # phases B+C: everything not covered by the ramp
            for t in range(TT):
                if t < RAMP_T:
                    xt = xts[t]
                    jlist = (1, 2)
                else:
                    xt = xin.tile([128, KT, 128], BF16, tag="xt", name="xt")
                    nc.sync.dma_start(out=xt, in_=x_d[t])
                    jlist = (0, 1, 2)
                for j in jlist:
                    o0, ow = O_SLICES[j]
                    pt = psum.tile([128, 512], F32, tag=f"pt{j}", name=f"pt{j}")
                    for k in range(KT):
                        nc.tensor.matmul(
                            pt[:, :ow],
                            xt[:, k, :],
                            w_sb[:, k, o0 : o0 + ow],
                            start=(k == 0),
                            stop=(k == KT - 1),
                        )
                    epilogue(pt, t, j)

    nc.compile()
    return nc


def _get_nc(uniform_thr):
    key = ("u" if uniform_thr else "g")
    if key not in _CACHE:
        _CACHE[key] = _build_nc(uniform_thr)
    return _CACHE[key]


def _run(inputs, trace=False, tmpdir=None):
    from concourse.bass_utils import run_bass_kernel_spmd

    x = np.asarray(inputs["input"], dtype=np.float32)
    w = np.asarray(inputs["weight"], dtype=np.float32)
    scale = np.asarray(inputs["scale"], dtype=np.float32)
    thr = np.asarray(inputs["threshold"], dtype=np.float32).reshape(O)
    bias = np.asarray(inputs["bias"], dtype=np.float32)

    uniform_thr = bool(np.all(thr == thr[0]))

    # [T, I] -> [TT, p=128(feat), KT, tl=128(tok)] bf16, contiguous per partition
    x2 = x.reshape(T, I).astype(ml_dtypes.bfloat16)
    xh = np.ascontiguousarray(x2.reshape(TT, 128, KT, 128).transpose(0, 3, 2, 1))

    in_maps = []
    for c in range(NCORES):
        sl = slice(c * OSH, (c + 1) * OSH)
        wh = np.ascontiguousarray(w[sl].T).reshape(KT, 128, OSH)
        tv = np.empty((128, 4), dtype=np.float32)
        tv[:, 0] = -thr[sl][0]
        tv[:, 1] = thr[sl][0]
        tv[:, 2] = thr[sl][0]
        tv[:, 3] = -thr[sl][0]
        in_maps.append(
            {
                "x": xh,
                "w": wh,
                "thrv": tv,
                "thr": np.ascontiguousarray(np.broadcast_to(thr[sl], (128, OSH))),
                "scale": np.ascontiguousarray(np.broadcast_to(scale[sl], (128, OSH))),
                "bias": np.ascontiguousarray(np.broadcast_to(bias[sl], (128, OSH))),
            }
        )

    nc = _get_nc(uniform_thr)
    res = run_bass_kernel_spmd(
        nc, in_maps, list(range(NCORES)), trace=trace, tmpdir=tmpdir
    )
    out = np.concatenate([res.results[c]["out"] for c in range(NCORES)], axis=1)
    return out.reshape(B, S, O), res


def kernel(**inputs) -> np.ndarray:
    out, _ = _run(inputs, trace=False)
    return out
